# revision 1
# baseline (speedup 1.0000x reference)
"""Trainium2 Bass kernel for LoraLinear:
    out = x @ W^T + 2.0 * (x @ A^T) @ B^T
    x: [4, 2048, 4096] f32, W: [4096, 4096], A: [64, 4096], B: [4096, 64]

The LoRA update is folded into the weight on the host (merged-LoRA
inference): out = x @ (W + 2*B@A)^T, exactly. The device then runs a pure
[8192 x 4096] @ [4096 x 4096] GEMM.

Sharding across 8 NeuronCores: 4-way data-parallel over tokens x 2-way
tensor-parallel over out-features. Each core computes a [2048 x 2048]
output block. No collectives; the host scatters shards and gathers blocks.

Per-core device program (SPMD, same program on all 8 cores):
  - The merged W'^T shard ([4096 x 2048] fp16, 16.8 MB) loads once on the
    SP DMA queue and stays resident in SBUF.
  - x^T streams once on the ACT DMA queue in 8 groups of 256 tokens, each
    group as 8 chunked DMAs aligned with k-blocks so compute can chase
    the transfers.
  - Per 128-token tile and 512-wide out-feature tile: 32 accumulating
    matmuls into one PSUM bank, DVE copy to SBUF, store on the SP queue.
  - Startup: the first group's matmuls run k-OUTER across all 8 PSUM
    banks (2 token tiles x 4 o-tiles = ~1.75us of PE work per W block),
    consuming W'^T blocks as they arrive from HBM (~1.5us/block) instead
    of stalling until the full weight is resident.

Matmuls run in fp16 (inputs host-cast; same PE rate as bf16, 8x finer
mantissa); accumulation is fp32 in PSUM. All DMAs are simple 2D
transfers - HWDGE queue fanout for 3D shapes breaks Tile's semaphore
accounting on this stack (sim race detector confirms).
"""

import numpy as np

import concourse.mybir as mybir
import concourse.tile as tile
from concourse import bacc
from concourse.bass_utils import run_bass_kernel_spmd

# problem dims (hardcoded per harness contract)
B, S, D_IN, D_OUT, R = 4, 2048, 4096, 4096, 64
SCALING = 2.0

T_TOTAL = B * S  # 8192 tokens
DP, TP = 4, 2  # token-parallel x feature-parallel over 8 cores
T_CORE = T_TOTAL // DP  # 2048
O_CORE = D_OUT // TP  # 2048
K = D_IN  # 4096

P = 128  # SBUF partitions / matmul contraction tile
KT = K // P  # 32 k-tiles
TG_W = 2 * P  # tokens per x group (2 token tiles)
TG = T_CORE // TG_W  # 8 groups per core
NO = 512  # matmul moving free dim (one PSUM bank of fp32)
OT = O_CORE // NO  # 4 out-feature tiles per core
X_CHUNKS = 16  # DMAs per x group, each covering 2 k-blocks

MM_DT = mybir.dt.float16
MM_NP = np.float16
F32 = mybir.dt.float32

_NC_CACHE = {}


def _build_program():
    nc = bacc.Bacc()
    # xq[g][p][kt*256+u] = x^T[kt*128+p, g*256+u]  (host pre-arranged)
    xq = nc.declare_dram_parameter("xq", [TG, P, KT * TG_W], MM_DT, isOutput=False)
    wt = nc.declare_dram_parameter("wt", [K, O_CORE], MM_DT, isOutput=False)
    out = nc.declare_dram_parameter("out", [T_CORE, O_CORE], F32, isOutput=True)

    with tile.TileContext(nc) as tc:
        with (
            tc.tile_pool(name="wres", bufs=1) as wres,
            tc.tile_pool(name="xin", bufs=2) as xin,
            tc.tile_pool(name="ostage", bufs=4) as ostage,
            tc.tile_pool(name="psacc", bufs=8, space="PSUM") as psacc,
        ):
            # resident W'^T as 32 k-blocks side by side -> [128, 32*2048].
            # Split across BOTH HWDGE queues (even k on SP, odd k on ACT,
            # interleaved with g0's x chunks) so the early weight stream is
            # not capped by one queue's descriptor ramp.
            wtile = wres.tile([P, KT * O_CORE], MM_DT, name="wtile")
            wt_r = wt[:].rearrange("(kt p) o -> kt p o", p=P)

            xtiles = {}
            chunk = KT * TG_W // X_CHUNKS

            def w_dma(eng, k):
                eng.dma_start(
                    out=wtile[:, k * O_CORE : (k + 1) * O_CORE], in_=wt_r[k]
                )

            def load_x(g, after=None):
                """after: instruction the first chunk DMA waits for —
                throttles prefetch off the HBM while W is the critical stream.
                Returns the chunk DMA instructions (for post-hoc pacing)."""
                xt_ = xin.tile([P, KT * TG_W], MM_DT, name="xtile", tag="xtile")
                dmas = []
                for c in range(X_CHUNKS):
                    dma = nc.scalar.dma_start(
                        out=xt_[:, c * chunk : (c + 1) * chunk],
                        in_=xq[g][:, c * chunk : (c + 1) * chunk],
                    )
                    if after is not None and c == 0:
                        tile.add_dep_helper(
                            dma.ins, after.ins, reason="x prefetch throttle"
                        )
                    dmas.append(dma)
                xtiles[g] = xt_
                return dmas

            def x_slice(g, j, k):
                """lhsT for token tile j (0/1) of group g, k-block k."""
                return xtiles[g][:, k * TG_W + j * P : k * TG_W + j * P + P]

            def w_slice(k, o):
                return wtile[:, k * O_CORE + o * NO : k * O_CORE + o * NO + NO]

            def finish_tile(g, j, o, ps):
                osb = ostage.tile([P, NO], F32, name="osb")
                nc.vector.tensor_copy(osb[:], ps[:])
                t = g * 2 + j
                nc.sync.dma_start(
                    out=out[t * P : (t + 1) * P, o * NO : (o + 1) * NO],
                    in_=osb[:],
                )

            def base_pass(g, j, o):
                ps = psacc.tile([P, NO], F32, name="ps", tag="ps")
                for k in range(KT):
                    nc.tensor.matmul(
                        ps[:],
                        x_slice(g, j, k),
                        w_slice(k, o),
                        start=(k == 0),
                        stop=(k == KT - 1),
                    )
                finish_tile(g, j, o, ps)

            # --- startup: consume W blocks AS THEY ARRIVE, k-outer over all
            # 8 PSUM banks so each block gets ~1.75us of PE work vs ~1.5us
            # arrival, instead of stalling until the full W is resident.
            # Both HWDGE queues carry the startup stream in exact consumption
            # order, balanced: per k-block, the 128 KB x slice then the 512 KB
            # W block, alternating queues by k parity.
            xt0 = xin.tile([P, KT * TG_W], MM_DT, name="xtile", tag="xtile")
            for k in range(KT):
                eng = nc.sync if k % 2 == 0 else nc.scalar
                eng.dma_start(
                    out=xt0[:, k * TG_W : (k + 1) * TG_W],
                    in_=xq[0][:, k * TG_W : (k + 1) * TG_W],
                )
                w_dma(eng, k)
            xtiles[0] = xt0
            start_ps = {
                (j, o): psacc.tile([P, NO], F32, name="ps", tag="ps")
                for j in range(2)
                for o in range(OT)
            }
            k_mms = {}
            for k in range(KT):
                for o in range(OT):
                    for j in range(2):
                        mm = nc.tensor.matmul(
                            start_ps[j, o][:],
                            x_slice(0, j, k),
                            w_slice(k, o),
                            start=(k == 0),
                            stop=(k == KT - 1),
                        )
                        if j == 0 and o == 0:
                            k_mms[k] = mm
            for j in range(2):
                for o in range(OT):
                    finish_tile(0, j, o, start_ps[j, o])

            # --- steady state (x loads queue naturally behind the W-odd
            # blocks on the ACT queue) ---
            for g in range(1, TG):
                load_x(g)
                for j in range(2):
                    for o in range(OT):
                        base_pass(g, j, o)
    return nc


def _get_program():
    if "nc" not in _NC_CACHE:
        nc = _build_program()
        nc.finalize()  # runs Bacc.compile(): reg alloc, event-sem wait splitting
        _NC_CACHE["nc"] = nc
    return _NC_CACHE["nc"]


def _prep_x_shard(xs):
    """[T_CORE, K] f32 -> [TG, P, KT*TG_W] fp16,
    xq[g,p,kt*256+u] = xs[g*256+u, kt*128+p]."""
    x4 = xs.reshape(TG, TG_W, KT, P)  # [g, u, kt, p]
    return (
        np.ascontiguousarray(x4.transpose(0, 3, 2, 1))
        .astype(MM_NP)
        .reshape(TG, P, KT * TG_W)
    )


def _prep_in_maps(x, weight, lora_A, lora_B):
    xf = np.ascontiguousarray(x.reshape(T_TOTAL, K))

    # merged-LoRA weight, computed in fp32 on host: W' = W + 2*B@A
    w_merged = weight + SCALING * (lora_B @ lora_A)

    xq_shards = [_prep_x_shard(xf[d * T_CORE : (d + 1) * T_CORE]) for d in range(DP)]
    wt_shards = [
        np.ascontiguousarray(w_merged[tp * O_CORE : (tp + 1) * O_CORE].T).astype(MM_NP)
        for tp in range(TP)
    ]

    in_maps = []
    for core in range(8):
        d, tp = core // TP, core % TP
        in_maps.append({"xq": xq_shards[d], "wt": wt_shards[tp]})
    return in_maps


def _gather(results):
    out = np.empty((T_TOTAL, D_OUT), dtype=np.float32)
    for core in range(8):
        d, tp = core // TP, core % TP
        out[d * T_CORE : (d + 1) * T_CORE, tp * O_CORE : (tp + 1) * O_CORE] = results[
            core
        ]["out"]
    return out.reshape(B, S, D_OUT)


def run(x, weight, lora_A, lora_B, trace=False):
    """Returns (output, BassKernelResults)."""
    nc = _get_program()
    in_maps = _prep_in_maps(
        np.asarray(x, dtype=np.float32),
        np.asarray(weight, dtype=np.float32),
        np.asarray(lora_A, dtype=np.float32),
        np.asarray(lora_B, dtype=np.float32),
    )
    res = run_bass_kernel_spmd(nc, in_maps, list(range(8)), trace=trace)
    return _gather(res.results), res


def kernel(x, weight, lora_A, lora_B):
    out, _ = run(x, weight, lora_A, lora_B, trace=False)
    return out



# revision 2
# speedup vs baseline: 1.0061x; 1.0061x over previous
"""Trainium2 Bass kernel for LoraLinear:
    out = x @ W^T + 2.0 * (x @ A^T) @ B^T
    x: [4, 2048, 4096] f32, W: [4096, 4096], A: [64, 4096], B: [4096, 64]

The LoRA update is folded into the weight on the host (merged-LoRA
inference): out = x @ (W + 2*B@A)^T, exactly. The device then runs a pure
[8192 x 4096] @ [4096 x 4096] GEMM in fp16 (fp32 PSUM accumulation).

Sharding across 8 NeuronCores: 8-way data-parallel over tokens. Each core
computes out[d*1024:(d+1)*1024, :] = x_shard @ W'^T with the FULL merged
weight streamed from HBM (33.6 MB fp16 at a leisurely ~77 GB/s) and its
1024-token x^T shard RESIDENT in SBUF (8.4 MB). No collectives.

Why this layout: the PE stream (2048 N=512 matmuls x 216 ns = 442.8 us)
is the roofline; everything else must hide behind it. Keeping x resident
and streaming W k-chunk-by-k-chunk makes the startup requirement tiny
(first matmul needs one 256 KB x chunk + one 128 KB W chunk) instead of
a full resident-weight load, and the steady state needs only ~77 GB/s.

Per-core program:
  - 8 warm-up matmuls on a zeroed scratch tile run during the DMA ramp so
    the PE HAM clock-gate reaches 8/8 (2.4 GHz) before real work arrives.
  - o-group 0 (first 512 out-features) runs k-OUTER over all 8 PSUM banks
    (8 token tiles): per 128-deep k-step the core consumes one 256 KB x
    chunk + one 128 KB W chunk per 1.71 us of PE work (~225 GB/s), which
    the two HWDGE queues sustain from the first chunk on. This pass also
    doubles as the x residency load.
  - o-groups 1..7 run j-OUTER/k-inner: each token tile's 32-matmul chain
    finishes 6.9 us apart, so the PSUM->SBUF copies and output stores are
    evenly spaced (no evacuation burst, no PSUM-reuse stall) and the tail
    after the very last matmul is one copy + one store.
  - The last tile's evacuation is split in half across DVE and ACT with
    two half-stores on separate DMA queues to shorten the drain chain.
"""

import numpy as np

import concourse.mybir as mybir
import concourse.tile as tile
from concourse import bacc
from concourse.bass_utils import run_bass_kernel_spmd

# problem dims (hardcoded per harness contract)
B, S, D_IN, D_OUT, R = 4, 2048, 4096, 4096, 64
SCALING = 2.0

T_TOTAL = B * S  # 8192 tokens
NCORES = 8
T_CORE = T_TOTAL // NCORES  # 1024 tokens per core
K = D_IN  # 4096

P = 128  # SBUF partitions / matmul contraction tile
KT = K // P  # 32 k-tiles
JT = T_CORE // P  # 8 token tiles per core
NO = 512  # matmul moving free dim (one PSUM bank of fp32)
OG = D_OUT // NO  # 8 out-feature groups

MM_DT = mybir.dt.float16
MM_NP = np.float16
F32 = mybir.dt.float32

_NC_CACHE = {}


def _build_program():
    nc = bacc.Bacc()
    # xq[p, k*1024 + j*128 + u] = x^T[k*128+p, j*128+u]  (host pre-arranged)
    xq = nc.declare_dram_parameter("xq", [P, KT * JT * P], MM_DT, isOutput=False)
    # wq[og][p, k*512 + c] = W'^T[k*128+p, og*512+c]
    wq = nc.declare_dram_parameter("wq", [OG, P, KT * NO], MM_DT, isOutput=False)
    out = nc.declare_dram_parameter("out", [T_CORE, D_OUT], F32, isOutput=True)

    with tile.TileContext(nc) as tc:
        with (
            tc.tile_pool(name="xres", bufs=1) as xres,
            tc.tile_pool(name="wring", bufs=2) as wring,
            tc.tile_pool(name="ostage", bufs=4) as ostage,
            tc.tile_pool(name="warm", bufs=1) as warm,
            tc.tile_pool(name="psacc", bufs=8, space="PSUM") as psacc,
        ):
            # --- PE warm-up: 8 N=512 matmuls on zeroed scratch keep the PE
            # busy from ~6.5us so the HAM clock-gate is at 8/8 by the time
            # the first real operands land (~9us).
            scratch = warm.tile([P, NO], MM_DT, name="scratch")
            nc.gpsimd.memset(scratch[:], 0.0)
            ps_warm = psacc.tile([P, NO], F32, name="ps", tag="ps")
            for _ in range(8):
                nc.tensor.matmul(
                    ps_warm[:], scratch[:, :P], scratch[:], start=True, stop=True
                )

            def x_tile(xt, j, k):
                """stationary lhsT for token tile j, k-block k."""
                base = k * (JT * P) + j * P
                return xt[:, base : base + P]

            def w_chunk(wt, k):
                return wt[:, k * NO : (k + 1) * NO]

            def store(og, j, osb, half=None):
                dst = out[j * P : (j + 1) * P, og * NO : (og + 1) * NO]
                if half is None:
                    nc.sync.dma_start(out=dst, in_=osb[:])
                elif half == 0:
                    nc.sync.dma_start(out=dst[:, : NO // 2], in_=osb[:, : NO // 2])
                else:
                    nc.scalar.dma_start(out=dst[:, NO // 2 :], in_=osb[:, NO // 2 :])

            # --- o-group 0: x residency load + k-outer compute ---
            # x chunk k and W chunk k alternate between the two HWDGE
            # queues so each carries ~112 GB/s while the PE consumes one
            # (x,W) chunk pair per 1.71us.
            xt = xres.tile([P, KT * JT * P], MM_DT, name="xtile")
            w0 = wring.tile([P, KT * NO], MM_DT, name="wtile", tag="w")
            for k in range(KT):
                qx = nc.sync if k % 2 == 0 else nc.scalar
                qw = nc.scalar if k % 2 == 0 else nc.sync
                xcol = slice(k * JT * P, (k + 1) * JT * P)
                qx.dma_start(out=xt[:, xcol], in_=xq[:, xcol])
                qw.dma_start(out=w_chunk(w0, k), in_=wq[0][:, k * NO : (k + 1) * NO])

            ps0 = {
                j: psacc.tile([P, NO], F32, name="ps", tag="ps") for j in range(JT)
            }
            for k in range(KT):
                for j in range(JT):
                    nc.tensor.matmul(
                        ps0[j][:],
                        x_tile(xt, j, k),
                        w_chunk(w0, k),
                        start=(k == 0),
                        stop=(k == KT - 1),
                    )
            for j in range(JT):
                osb = ostage.tile([P, NO], F32, name="osb")
                nc.vector.tensor_copy(osb[:], ps0[j][:])
                store(0, j, osb)

            # --- o-groups 1..7: j-outer / k-inner on prefetched W ---
            wt_cur = w0
            wt_next = None
            for og in range(1, OG):
                # prefetch this group's W (triggers queue on the ACT HWDGE
                # behind o-group 0's interleaved stream; the ring pool's
                # bufs=2 paces it one group ahead of consumption)
                if og == 1:
                    wt_next = wring.tile([P, KT * NO], MM_DT, name="wtile", tag="w")
                    for k in range(KT):
                        nc.scalar.dma_start(
                            out=w_chunk(wt_next, k),
                            in_=wq[1][:, k * NO : (k + 1) * NO],
                        )
                wt_cur, wt_next = wt_next, None
                if og + 1 < OG:
                    wt_next = wring.tile([P, KT * NO], MM_DT, name="wtile", tag="w")
                    for k in range(KT):
                        nc.scalar.dma_start(
                            out=w_chunk(wt_next, k),
                            in_=wq[og + 1][:, k * NO : (k + 1) * NO],
                        )
                for j in range(JT):
                    ps = psacc.tile([P, NO], F32, name="ps", tag="ps")
                    for k in range(KT):
                        nc.tensor.matmul(
                            ps[:],
                            x_tile(xt, j, k),
                            w_chunk(wt_cur, k),
                            start=(k == 0),
                            stop=(k == KT - 1),
                        )
                    osb = ostage.tile([P, NO], F32, name="osb")
                    last = og == OG - 1 and j == JT - 1
                    if last:
                        # split the final evacuation DVE/ACT + two queues
                        # to shorten the post-matmul drain chain
                        nc.vector.tensor_copy(osb[:, : NO // 2], ps[:, : NO // 2])
                        nc.scalar.copy(osb[:, NO // 2 :], ps[:, NO // 2 :])
                        store(og, j, osb, half=0)
                        store(og, j, osb, half=1)
                    else:
                        nc.vector.tensor_copy(osb[:], ps[:])
                        store(og, j, osb)
    return nc


def _get_program():
    if "nc" not in _NC_CACHE:
        nc = _build_program()
        nc.finalize()  # runs Bacc.compile(): reg alloc, event-sem wait splitting
        _NC_CACHE["nc"] = nc
    return _NC_CACHE["nc"]


def _prep_in_maps(x, weight, lora_A, lora_B):
    xf = np.ascontiguousarray(x.reshape(T_TOTAL, K))

    # merged-LoRA weight, computed in fp32 on host: W' = W + 2*B@A
    w_merged = weight + SCALING * (lora_B @ lora_A)

    # wq[og, p, k*512+c] = W'[og*512+c, k*128+p]
    w4 = w_merged.reshape(OG, NO, KT, P)  # [og, c, k, p]
    wq = np.ascontiguousarray(w4.transpose(0, 3, 2, 1)).astype(MM_NP)
    wq = wq.reshape(OG, P, KT * NO)

    in_maps = []
    for d in range(NCORES):
        xs = xf[d * T_CORE : (d + 1) * T_CORE]  # [1024, 4096]
        # xq[p, k*1024 + j*128 + u] = xs[j*128+u, k*128+p]
        x4 = xs.reshape(JT, P, KT, P)  # [j, u, k, p]
        xqd = np.ascontiguousarray(x4.transpose(3, 2, 0, 1)).astype(MM_NP)
        in_maps.append({"xq": xqd.reshape(P, KT * JT * P), "wq": wq})
    return in_maps


def _gather(results):
    out = np.empty((T_TOTAL, D_OUT), dtype=np.float32)
    for d in range(NCORES):
        out[d * T_CORE : (d + 1) * T_CORE] = results[d]["out"]
    return out.reshape(B, S, D_OUT)


def run(x, weight, lora_A, lora_B, trace=False):
    """Returns (output, BassKernelResults)."""
    nc = _get_program()
    in_maps = _prep_in_maps(
        np.asarray(x, dtype=np.float32),
        np.asarray(weight, dtype=np.float32),
        np.asarray(lora_A, dtype=np.float32),
        np.asarray(lora_B, dtype=np.float32),
    )
    res = run_bass_kernel_spmd(nc, in_maps, list(range(8)), trace=trace)
    return _gather(res.results), res


def kernel(x, weight, lora_A, lora_B):
    out, _ = run(x, weight, lora_A, lora_B, trace=False)
    return out


# revision 4
# speedup vs baseline: 1.0095x; 1.0034x over previous
"""Trainium2 Bass kernel for LoraLinear:
    out = x @ W^T + 2.0 * (x @ A^T) @ B^T
    x: [4, 2048, 4096] f32, W: [4096, 4096], A: [64, 4096], B: [4096, 64]

The LoRA update is folded into the weight on the host (merged-LoRA
inference): out = x @ (W + 2*B@A)^T, exactly. The device then runs a pure
[8192 x 4096] @ [4096 x 4096] GEMM in fp16 (fp32 PSUM accumulation).

Sharding across 8 NeuronCores: 8-way data-parallel over tokens. Each core
computes out[d*1024:(d+1)*1024, :] = x_shard @ W'^T with the FULL merged
weight streamed from HBM (33.6 MB fp16 at a leisurely ~77 GB/s) and its
1024-token x^T shard RESIDENT in SBUF (8.4 MB). No collectives.

Why this layout: the PE stream (2048 N=512 matmuls x 216 ns = 442.8 us)
is the roofline; everything else must hide behind it. Keeping x resident
and streaming W k-chunk-by-k-chunk makes the startup requirement tiny
(first matmul needs one 256 KB x chunk + one 128 KB W chunk) instead of
a full resident-weight load, and the steady state needs only ~77 GB/s.

Per-core program:
  - 8 warm-up matmuls on a zeroed scratch tile run during the DMA ramp so
    the PE HAM clock-gate reaches 8/8 (2.4 GHz) before real work arrives.
  - o-group 0 (first 512 out-features) runs k-OUTER over all 8 PSUM banks
    (8 token tiles): per 128-deep k-step the core consumes one 256 KB x
    chunk + one 128 KB W chunk per 1.71 us of PE work (~225 GB/s), which
    the two HWDGE queues sustain from the first chunk on. This pass also
    doubles as the x residency load.
  - o-groups 1..7 run j-OUTER/k-inner: each token tile's 32-matmul chain
    finishes 6.9 us apart, so the PSUM->SBUF copies and output stores are
    evenly spaced (no evacuation burst, no PSUM-reuse stall) and the tail
    after the very last matmul is one copy + one store.
  - The last tile's evacuation is split in half across DVE and ACT with
    two half-stores on separate DMA queues to shorten the drain chain.
"""

import numpy as np

import concourse.mybir as mybir
import concourse.tile as tile
from concourse import bacc
from concourse.bass_utils import run_bass_kernel_spmd

# problem dims (hardcoded per harness contract)
B, S, D_IN, D_OUT, R = 4, 2048, 4096, 4096, 64
SCALING = 2.0

T_TOTAL = B * S  # 8192 tokens
NCORES = 8
T_CORE = T_TOTAL // NCORES  # 1024 tokens per core
K = D_IN  # 4096

P = 128  # SBUF partitions / matmul contraction tile
KT = K // P  # 32 k-tiles
JT = T_CORE // P  # 8 token tiles per core
NO = 512  # matmul moving free dim (one PSUM bank of fp32)
OG = D_OUT // NO  # 8 out-feature groups

MM_DT = mybir.dt.float16
MM_NP = np.float16
F32 = mybir.dt.float32

_NC_CACHE = {}


def _build_program():
    nc = bacc.Bacc()
    # xq[p, k*1024 + j*128 + u] = x^T[k*128+p, j*128+u]  (host pre-arranged)
    xq = nc.declare_dram_parameter("xq", [P, KT * JT * P], MM_DT, isOutput=False)
    # wq[og][p, k*512 + c] = W'^T[k*128+p, og*512+c]
    wq = nc.declare_dram_parameter("wq", [OG, P, KT * NO], MM_DT, isOutput=False)
    out = nc.declare_dram_parameter("out", [T_CORE, D_OUT], F32, isOutput=True)

    with tile.TileContext(nc) as tc:
        with (
            tc.tile_pool(name="xres", bufs=1) as xres,
            tc.tile_pool(name="wring", bufs=2) as wring,
            tc.tile_pool(name="ostage", bufs=4) as ostage,
            tc.tile_pool(name="warm", bufs=1) as warm,
            tc.tile_pool(name="psacc", bufs=8, space="PSUM") as psacc,
        ):
            # --- PE warm-up: 8 N=512 matmuls on zeroed scratch keep the PE
            # busy from ~6.5us so the HAM clock-gate is at 8/8 by the time
            # the first real operands land (~9us).
            scratch = warm.tile([P, NO], MM_DT, name="scratch")
            nc.gpsimd.memset(scratch[:], 0.0)
            ps_warm = psacc.tile([P, NO], F32, name="ps", tag="ps")
            for _ in range(4):
                nc.tensor.matmul(
                    ps_warm[:], scratch[:, :P], scratch[:], start=True, stop=True
                )

            def x_tile(xt, j, k):
                """stationary lhsT for token tile j, k-block k."""
                base = k * (JT * P) + j * P
                return xt[:, base : base + P]

            def w_chunk(wt, k):
                return wt[:, k * NO : (k + 1) * NO]

            def store(og, j, osb, half=None):
                dst = out[j * P : (j + 1) * P, og * NO : (og + 1) * NO]
                if half is None:
                    nc.sync.dma_start(out=dst, in_=osb[:])
                elif half == 0:
                    nc.sync.dma_start(out=dst[:, : NO // 2], in_=osb[:, : NO // 2])
                else:
                    nc.scalar.dma_start(out=dst[:, NO // 2 :], in_=osb[:, NO // 2 :])

            # --- o-group 0: x residency load + k-outer compute ---
            # x chunk k and W chunk k alternate between the two HWDGE
            # queues so each carries ~112 GB/s while the PE consumes one
            # (x,W) chunk pair per 1.71us.
            xt = xres.tile([P, KT * JT * P], MM_DT, name="xtile")
            w0 = wring.tile([P, KT * NO], MM_DT, name="wtile", tag="w")
            for k in range(KT):
                qx = nc.sync if k % 2 == 0 else nc.scalar
                qw = nc.scalar if k % 2 == 0 else nc.sync
                xcol = slice(k * JT * P, (k + 1) * JT * P)
                qx.dma_start(out=xt[:, xcol], in_=xq[:, xcol])
                qw.dma_start(out=w_chunk(w0, k), in_=wq[0][:, k * NO : (k + 1) * NO])

            ps0 = {
                j: psacc.tile([P, NO], F32, name="ps", tag="ps") for j in range(JT)
            }
            for k in range(KT):
                for j in range(JT):
                    nc.tensor.matmul(
                        ps0[j][:],
                        x_tile(xt, j, k),
                        w_chunk(w0, k),
                        start=(k == 0),
                        stop=(k == KT - 1),
                    )
            for j in range(JT):
                osb = ostage.tile([P, NO], F32, name="osb")
                nc.vector.tensor_copy(osb[:], ps0[j][:])
                store(0, j, osb)

            # --- o-groups 1..7: j-outer / k-inner on prefetched W ---
            wt_cur = w0
            wt_next = None
            for og in range(1, OG):
                # prefetch this group's W (triggers queue on the ACT HWDGE
                # behind o-group 0's interleaved stream; the ring pool's
                # bufs=2 paces it one group ahead of consumption)
                if og == 1:
                    wt_next = wring.tile([P, KT * NO], MM_DT, name="wtile", tag="w")
                    for k in range(KT):
                        nc.scalar.dma_start(
                            out=w_chunk(wt_next, k),
                            in_=wq[1][:, k * NO : (k + 1) * NO],
                        )
                wt_cur, wt_next = wt_next, None
                if og + 1 < OG:
                    wt_next = wring.tile([P, KT * NO], MM_DT, name="wtile", tag="w")
                    for k in range(KT):
                        nc.scalar.dma_start(
                            out=w_chunk(wt_next, k),
                            in_=wq[og + 1][:, k * NO : (k + 1) * NO],
                        )
                for j in range(JT):
                    last = og == OG - 1 and j == JT - 1
                    if last:
                        # Final tile: accumulate the two 256-wide halves in
                        # two different PSUM banks so DVE and ACT can
                        # evacuate them in parallel (same-bank PSUM access
                        # by two engines is serialized by hardware), with
                        # the two half-stores on separate DMA queues. This
                        # shortens the post-last-matmul drain chain.
                        psA = psacc.tile([P, NO], F32, name="ps", tag="ps")
                        psB = psacc.tile([P, NO], F32, name="ps", tag="ps")
                        h = NO // 2
                        for k in range(KT):
                            wc = w_chunk(wt_cur, k)
                            nc.tensor.matmul(
                                psA[:, :h],
                                x_tile(xt, j, k),
                                wc[:, :h],
                                start=(k == 0),
                                stop=(k == KT - 1),
                            )
                            nc.tensor.matmul(
                                psB[:, :h],
                                x_tile(xt, j, k),
                                wc[:, h:],
                                start=(k == 0),
                                stop=(k == KT - 1),
                            )
                        osb = ostage.tile([P, NO], F32, name="osb")
                        nc.vector.tensor_copy(osb[:, :h], psA[:, :h])
                        nc.scalar.copy(osb[:, h:], psB[:, :h])
                        store(og, j, osb, half=0)
                        store(og, j, osb, half=1)
                    else:
                        ps = psacc.tile([P, NO], F32, name="ps", tag="ps")
                        for k in range(KT):
                            nc.tensor.matmul(
                                ps[:],
                                x_tile(xt, j, k),
                                w_chunk(wt_cur, k),
                                start=(k == 0),
                                stop=(k == KT - 1),
                            )
                        osb = ostage.tile([P, NO], F32, name="osb")
                        nc.vector.tensor_copy(osb[:], ps[:])
                        store(og, j, osb)
    return nc


def _get_program():
    if "nc" not in _NC_CACHE:
        nc = _build_program()
        nc.finalize()  # runs Bacc.compile(): reg alloc, event-sem wait splitting
        _NC_CACHE["nc"] = nc
    return _NC_CACHE["nc"]


def _prep_in_maps(x, weight, lora_A, lora_B):
    xf = np.ascontiguousarray(x.reshape(T_TOTAL, K))

    # merged-LoRA weight, computed in fp32 on host: W' = W + 2*B@A
    w_merged = weight + SCALING * (lora_B @ lora_A)

    # wq[og, p, k*512+c] = W'[og*512+c, k*128+p]
    w4 = w_merged.reshape(OG, NO, KT, P)  # [og, c, k, p]
    wq = np.ascontiguousarray(w4.transpose(0, 3, 2, 1)).astype(MM_NP)
    wq = wq.reshape(OG, P, KT * NO)

    in_maps = []
    for d in range(NCORES):
        xs = xf[d * T_CORE : (d + 1) * T_CORE]  # [1024, 4096]
        # xq[p, k*1024 + j*128 + u] = xs[j*128+u, k*128+p]
        x4 = xs.reshape(JT, P, KT, P)  # [j, u, k, p]
        xqd = np.ascontiguousarray(x4.transpose(3, 2, 0, 1)).astype(MM_NP)
        in_maps.append({"xq": xqd.reshape(P, KT * JT * P), "wq": wq})
    return in_maps


def _gather(results):
    out = np.empty((T_TOTAL, D_OUT), dtype=np.float32)
    for d in range(NCORES):
        out[d * T_CORE : (d + 1) * T_CORE] = results[d]["out"]
    return out.reshape(B, S, D_OUT)


def run(x, weight, lora_A, lora_B, trace=False):
    """Returns (output, BassKernelResults)."""
    nc = _get_program()
    in_maps = _prep_in_maps(
        np.asarray(x, dtype=np.float32),
        np.asarray(weight, dtype=np.float32),
        np.asarray(lora_A, dtype=np.float32),
        np.asarray(lora_B, dtype=np.float32),
    )
    res = run_bass_kernel_spmd(nc, in_maps, list(range(8)), trace=trace)
    return _gather(res.results), res


def kernel(x, weight, lora_A, lora_B):
    out, _ = run(x, weight, lora_A, lora_B, trace=False)
    return out
